# revision 16
# baseline (speedup 1.0000x reference)
"""Trainium2 Bass kernel for nn_Detection (retrieval_knn).

Math note: the reference builds an [N,N] pairwise-distance matrix and takes
``nn_idx = argmin(dist, axis=1)`` but then uses only ``nn_idx[0]`` — the
nearest neighbour of point 0. Row 0's distance to itself is exactly 0 (the
global minimum of that row; squared distances are computed exactly in int32),
and jnp.argmin tie-breaks to the first index, so ``nn_idx[0] == 0`` for every
possible input. The whole N^2 distance/argmin stage therefore reduces to
``neighbor_feat = relu(features[b, 0])`` and the per-batch score is

    f      = relu(features[b])                      # [N, C]
    w      = exp(-relu(features[b, 0]))             # [C]
    gamma  = max_c(f * exp(f) * w[c]) / max_c(f)    # [N]
    out    = gamma / ||gamma||_2

(f * exp(f) == relu(x) * exp(x), so relu and exp run on independent engines).

Sharding: 8 cores x 2048 rows (4 cores per batch), replicating each batch's
row-0 feature vector. Layout per core: SBUF [128 partitions, 512], partition
p holding rows 16p..16p+15 (16 segments of C=32).

TRN2 quirks found on hardware, baked in here:
 - tensor_reduce with a 3D (segmented) access pattern hangs the DVE; the
   segmented row-max is a 5-step halving tree of tensor_tensor(max) ops.
 - tensor_tensor is not a legal GPSIMD opcode; elementwise work stays on
   DVE/ACT.

Each core returns its 2048 gammas; the host applies the per-batch scalar
normalisation (gather + norm is the cross-shard epilogue).
"""

import numpy as np

B, N, C = 2, 8192, 32
N_CORES = 8
CORES_PER_BATCH = N_CORES // B          # 4
ROWS = N // CORES_PER_BATCH             # 2048 rows per core
P = 128                                 # SBUF partitions
G = ROWS // P                           # 16 row-segments per partition
F = G * C                               # 512 floats per partition

_CACHE = {}


def _build_nc():
    import concourse.tile as tile
    from concourse import bacc, mybir

    AF = mybir.ActivationFunctionType
    ALU = mybir.AluOpType

    nc = bacc.Bacc("TRN2", target_bir_lowering=False, debug=False)
    feat = nc.dram_tensor("feat", [P, F], mybir.dt.float32, kind="ExternalInput")
    f0b = nc.dram_tensor("f0b", [P, C], mybir.dt.float32, kind="ExternalInput")
    out_g = nc.dram_tensor("out_g", [P, G], mybir.dt.float32,
                           kind="ExternalOutput")

    def seg_max_tree(pool, src, name):
        """Max over innermost C=32 of [P, G, 32] via halving
        tensor_tensor(max) steps; returns a [P, G] tile."""
        cur, width = src, C
        while width > 1:
            half = width // 2
            nxt = pool.tile([P, G * half], mybir.dt.float32, tag=f"{name}{half}")
            cur3 = cur[:].rearrange("p (g c) -> p g c", c=width)
            nxt3 = nxt[:].rearrange("p (g c) -> p g c", c=half)
            nc.vector.tensor_tensor(nxt3, cur3[:, :, 0:half],
                                    cur3[:, :, half:width], ALU.max)
            cur, width = nxt, half
        return cur

    with tile.TileContext(nc) as tc:
        with tc.tile_pool(name="pool", bufs=1) as pool:
            # f0 arrives host-replicated across partitions: w = exp(-relu(f0))
            # needs only ACT — no gpsimd partition_broadcast (whose mandatory
            # engine drain costs 2.5-5us on the critical path).
            s_f0b = pool.tile([P, C], mybir.dt.float32)
            nc.sync.dma_start(s_f0b[:], f0b.ap())
            # split the feat load across two engines' DGE queues so the two
            # halves transfer concurrently (one engine = one queue pair)
            s_raw = pool.tile([P, F], mybir.dt.float32)
            H = F // 2
            nc.sync.dma_start(s_raw[:, 0:H], feat.ap()[:, 0:H])
            nc.scalar.dma_start(s_raw[:, H:F], feat.ap()[:, H:F])

            s_f0r = pool.tile([P, C], mybir.dt.float32)
            nc.scalar.activation(s_f0r[:], s_f0b[:], AF.Relu)

            # t2 = f * exp(f) * exp(-f0r) == relu(raw) * exp(raw - f0r):
            # fusing w into the exponent deletes the broadcast multiply and
            # the second f0 activation. d = raw - f0r (broadcast over the 16
            # segments) on DVE, e2 = exp(d) on ACT, f = relu(raw) on DVE.
            s_d = pool.tile([P, F], mybir.dt.float32)
            d_3d = s_d[:].rearrange("p (g c) -> p g c", c=C)
            raw_3d = s_raw[:].rearrange("p (g c) -> p g c", c=C)
            f0r_b = s_f0r[:].unsqueeze(1).broadcast_to([P, G, C])
            nc.vector.tensor_tensor(d_3d, raw_3d, f0r_b, ALU.subtract)
            s_e = pool.tile([P, F], mybir.dt.float32)
            nc.scalar.activation(s_e[:], s_d[:], AF.Exp)
            s_f = pool.tile([P, F], mybir.dt.float32)
            nc.vector.tensor_scalar_max(s_f[:], s_raw[:], 0.0)
            s_t2 = pool.tile([P, F], mybir.dt.float32)
            nc.vector.tensor_mul(s_t2[:], s_f[:], s_e[:])

            # segmented maxes via halving trees
            s_m = seg_max_tree(pool, s_t2, "m")
            s_rmax = seg_max_tree(pool, s_f, "r")

            # gamma = m / rmax
            s_rinv = pool.tile([P, G], mybir.dt.float32)
            nc.vector.reciprocal(s_rinv[:], s_rmax[:])
            s_g = pool.tile([P, G], mybir.dt.float32)
            nc.vector.tensor_mul(s_g[:], s_m[:], s_rinv[:])

            nc.sync.dma_start(out_g.ap(), s_g[:])

    nc.compile()
    return nc


def _get_nc():
    if "nc" not in _CACHE:
        _CACHE["nc"] = _build_nc()
    return _CACHE["nc"]


def _make_in_maps(features):
    in_maps = []
    for core in range(N_CORES):
        b = core // CORES_PER_BATCH
        r0 = (core % CORES_PER_BATCH) * ROWS
        in_maps.append({
            "feat": np.ascontiguousarray(
                features[b, r0:r0 + ROWS, :], dtype=np.float32
            ).reshape(P, F),
            "f0b": np.ascontiguousarray(np.broadcast_to(
                features[b, 0:1, :], (P, C)), dtype=np.float32),
        })
    return in_maps


def _run(features, **spmd_kwargs):
    from concourse.bass_utils import run_bass_kernel_spmd

    nc = _get_nc()
    res = run_bass_kernel_spmd(
        nc, _make_in_maps(features), list(range(N_CORES)), **spmd_kwargs,
    )

    out = np.empty((B, N), dtype=np.float32)
    for b in range(B):
        cores = range(b * CORES_PER_BATCH, (b + 1) * CORES_PER_BATCH)
        gamma = np.concatenate(
            [res.results[c]["out_g"].reshape(-1) for c in cores])   # [8192]
        norm = np.float32(np.sqrt((gamma.astype(np.float64) ** 2).sum()))
        out[b] = gamma / norm
    return out.reshape(-1), res


def kernel(coords=None, features=None, len_batch=None, **_unused):
    features = np.asarray(features, dtype=np.float32)
    assert features.shape == (B, N, C), features.shape
    out, _ = _run(features)
    return out


# revision 19
# speedup vs baseline: 1.0478x; 1.0478x over previous
"""Trainium2 Bass kernel for nn_Detection (retrieval_knn).

Math note: the reference builds an [N,N] pairwise-distance matrix and takes
``nn_idx = argmin(dist, axis=1)`` but then uses only ``nn_idx[0]`` — the
nearest neighbour of point 0. Row 0's distance to itself is exactly 0 (the
global minimum of that row; squared distances are computed exactly in int32),
and jnp.argmin tie-breaks to the first index, so ``nn_idx[0] == 0`` for every
possible input. The whole N^2 distance/argmin stage therefore reduces to
``neighbor_feat = relu(features[b, 0])`` and the per-batch score is

    f      = relu(features[b])                      # [N, C]
    w      = exp(-relu(features[b, 0]))             # [C]
    gamma  = max_c(f * exp(f) * w[c]) / max_c(f)    # [N]
    out    = gamma / ||gamma||_2

(f * exp(f) == relu(x) * exp(x), so relu and exp run on independent engines).

Sharding: 8 cores x 2048 rows (4 cores per batch), replicating each batch's
row-0 feature vector. Layout per core: SBUF [128 partitions, 512], partition
p holding rows 16p..16p+15 (16 segments of C=32).

TRN2 quirks found on hardware, baked in here:
 - tensor_reduce with a 3D (segmented) access pattern hangs the DVE; the
   segmented row-max is a 5-step halving tree of tensor_tensor(max) ops.
 - tensor_tensor is not a legal GPSIMD opcode; elementwise work stays on
   DVE/ACT.

Each core returns its 2048 gammas; the host applies the per-batch scalar
normalisation (gather + norm is the cross-shard epilogue).
"""

import numpy as np

B, N, C = 2, 8192, 32
N_CORES = 8
CORES_PER_BATCH = N_CORES // B          # 4
ROWS = N // CORES_PER_BATCH             # 2048 rows per core
P = 128                                 # SBUF partitions
G = ROWS // P                           # 16 row-segments per partition
F = G * C                               # 512 floats per partition

_CACHE = {}


def _build_nc():
    import concourse.tile as tile
    from concourse import bacc, mybir

    AF = mybir.ActivationFunctionType
    ALU = mybir.AluOpType

    nc = bacc.Bacc("TRN2", target_bir_lowering=False, debug=False)
    feat = nc.dram_tensor("feat", [P, F], mybir.dt.float32, kind="ExternalInput")
    f0b = nc.dram_tensor("f0b", [P, C], mybir.dt.float32, kind="ExternalInput")
    out_g = nc.dram_tensor("out_g", [P, G], mybir.dt.float32,
                           kind="ExternalOutput")

    def seg_max_tree(pool, src, name):
        """Max over innermost C=32 of [P, G, 32] via halving
        tensor_tensor(max) steps; returns a [P, G] tile."""
        cur, width = src, C
        while width > 1:
            half = width // 2
            nxt = pool.tile([P, G * half], mybir.dt.float32, tag=f"{name}{half}")
            cur3 = cur[:].rearrange("p (g c) -> p g c", c=width)
            nxt3 = nxt[:].rearrange("p (g c) -> p g c", c=half)
            nc.vector.tensor_tensor(nxt3, cur3[:, :, 0:half],
                                    cur3[:, :, half:width], ALU.max)
            cur, width = nxt, half
        return cur

    with tile.TileContext(nc) as tc:
        with tc.tile_pool(name="pool", bufs=1) as pool:
            # f0 arrives host-replicated across partitions: w = exp(-relu(f0))
            # needs only ACT — no gpsimd partition_broadcast (whose mandatory
            # engine drain costs 2.5-5us on the critical path).
            s_f0b = pool.tile([P, C], mybir.dt.float32)
            nc.sync.dma_start(s_f0b[:], f0b.ap())
            s_raw = pool.tile([P, F], mybir.dt.float32)
            nc.sync.dma_start(s_raw[:], feat.ap())

            s_f0r = pool.tile([P, C], mybir.dt.float32)
            nc.scalar.activation(s_f0r[:], s_f0b[:], AF.Relu)

            # t2 = f * exp(f) * exp(-f0r) == relu(raw) * exp(raw - f0r):
            # fusing w into the exponent deletes the broadcast multiply and
            # the second f0 activation. d = raw - f0r (broadcast over the 16
            # segments) on DVE, e2 = exp(d) on ACT, f = relu(raw) on DVE.
            s_d = pool.tile([P, F], mybir.dt.float32)
            d_3d = s_d[:].rearrange("p (g c) -> p g c", c=C)
            raw_3d = s_raw[:].rearrange("p (g c) -> p g c", c=C)
            f0r_b = s_f0r[:].unsqueeze(1).broadcast_to([P, G, C])
            nc.vector.tensor_tensor(d_3d, raw_3d, f0r_b, ALU.subtract)
            s_e = pool.tile([P, F], mybir.dt.float32)
            nc.scalar.activation(s_e[:], s_d[:], AF.Exp)
            s_f = pool.tile([P, F], mybir.dt.float32)
            nc.vector.tensor_scalar_max(s_f[:], s_raw[:], 0.0)
            s_t2 = pool.tile([P, F], mybir.dt.float32)
            nc.vector.tensor_mul(s_t2[:], s_f[:], s_e[:])

            # segmented maxes via halving trees
            s_m = seg_max_tree(pool, s_t2, "m")
            s_rmax = seg_max_tree(pool, s_f, "r")

            # gamma = m / rmax
            s_rinv = pool.tile([P, G], mybir.dt.float32)
            nc.vector.reciprocal(s_rinv[:], s_rmax[:])
            s_g = pool.tile([P, G], mybir.dt.float32)
            nc.vector.tensor_mul(s_g[:], s_m[:], s_rinv[:])

            nc.sync.dma_start(out_g.ap(), s_g[:])

    nc.compile()
    return nc


def _get_nc():
    if "nc" not in _CACHE:
        _CACHE["nc"] = _build_nc()
    return _CACHE["nc"]


def _make_in_maps(features):
    in_maps = []
    for core in range(N_CORES):
        b = core // CORES_PER_BATCH
        r0 = (core % CORES_PER_BATCH) * ROWS
        in_maps.append({
            "feat": np.ascontiguousarray(
                features[b, r0:r0 + ROWS, :], dtype=np.float32
            ).reshape(P, F),
            "f0b": np.ascontiguousarray(np.broadcast_to(
                features[b, 0:1, :], (P, C)), dtype=np.float32),
        })
    return in_maps


def _run(features, **spmd_kwargs):
    from concourse.bass_utils import run_bass_kernel_spmd

    nc = _get_nc()
    res = run_bass_kernel_spmd(
        nc, _make_in_maps(features), list(range(N_CORES)), **spmd_kwargs,
    )

    out = np.empty((B, N), dtype=np.float32)
    for b in range(B):
        cores = range(b * CORES_PER_BATCH, (b + 1) * CORES_PER_BATCH)
        gamma = np.concatenate(
            [res.results[c]["out_g"].reshape(-1) for c in cores])   # [8192]
        norm = np.float32(np.sqrt((gamma.astype(np.float64) ** 2).sum()))
        out[b] = gamma / norm
    return out.reshape(-1), res


def kernel(coords=None, features=None, len_batch=None, **_unused):
    features = np.asarray(features, dtype=np.float32)
    assert features.shape == (B, N, C), features.shape
    out, _ = _run(features)
    return out
